# revision 34
# baseline (speedup 1.0000x reference)
"""CapsuleLayer (dynamic routing) Trainium2 Bass kernel.

Problem: x [64, 2048, 16], W [32, 2048, 32, 16] ->
  hat = einsum('bik,jidk->bijd', x, W); 3 routing iterations -> v [64, 32, 32].

Strategy (8 NeuronCores, In=2048 sharded 256/core; W never replicated):
  * hat is NEVER materialized. Three matmul families, all dense K=128:
      pass0:  sT0 = (1/Nc) * sum_i hat directly in [jd, b] layout
              (stationary = resident Wt chunk, moving = pre-scaled xT)
      (A):    agreement logits b += sum_d hat*v computed as
              G = (W . v) on PE (v folded into W), then mult+reduce vs x
      (B):    s = sum_i c*hat computed as xc = c*x, DMA-transpose to
              (k,i)-partition layout, PE contraction vs resident Wt
  * Wt (km-major W) is SBUF-RESIDENT (loaded once, 8MB); only Wd
    streams from DRAM (2x8MB, one per (A) pass).
  * elementwise work is row-balanced across DVE (2x bf16), Pool
    (gpsimd, ~3.8x slower/row) and ACT (PSUM evacuation + exp): Pool
    takes the r=3 quarter of (A) prod/tree and 3/16 of each xc unit.
  * s partials are AllReduced in bf16, split in two 4-slot halves that
    pipeline against squash and the neighbouring phases; exp + Z
    partial sums run per-ga inside (A) so only ga=3's tail is serial.
  * bf16 operands, fp32 PSUM accumulation / squash scalars.

Layout conventions (per core, i_loc in [0,256)):
  j-map:   r = j%4, c2 = (j//4)%2, ga = j//8, jj = 4*(j//8)+j%4, slot = j//4
  ik-major ((A) path):      ik = i_loc*16 + k
  km-major ((B)/pass0):     km = k*256 + i_loc; K-tile t2 = km//128
  sT/vT canonical:          sT[32*(j%4)+d, j//4, b]
  logits:                   [(c2,b) partition, ga, r, i_loc]
"""
import sys

if "/opt/trn_rl_repo" not in sys.path:
    sys.path.insert(0, "/opt/trn_rl_repo")

from contextlib import ExitStack

import ml_dtypes
import numpy as np

import concourse.mybir as mybir
import concourse.tile as tile
from concourse import bacc
from concourse.bass_utils import run_bass_kernel_spmd

B, In, Din, Nc, Dc = 64, 2048, 16, 32, 32
NCORES = 8
# (A) units (ga, cha) in this set read G straight from PSUM on DVE
# instead of the ACT evacuation path; tunes the ACT/DVE balance of (A).
DIRECT_UNITS = {(1, 2), (2, 2), (3, 2), (2, 1)}
IL = In // NCORES  # 256
EPS = 1e-7
FP32 = mybir.dt.float32
BF16 = mybir.dt.bfloat16

_KM_K = np.arange(4096) // 256   # km-major: k index
_KM_I = np.arange(4096) % 256    # km-major: i_loc index
_IK_I = np.arange(4096) // 16    # ik-major: i_loc index
_IK_K = np.arange(4096) % 16     # ik-major: k index


def _host_prep_core(x, W, core):
    """Build per-core input arrays. x, W are the full fp32 inputs."""
    i0 = core * IL
    Wc = np.ascontiguousarray(W[:, i0:i0 + IL])        # [Nc, IL, Dc, Din]
    xc = np.ascontiguousarray(x[:, i0:i0 + IL])        # [B, IL, Din]

    km = np.arange(128)[:, None] + 128 * np.arange(32)[None, :]   # [128, 32]
    ikm, kkm = _KM_I[km], _KM_K[km]

    # Wt [128, 32, 1024] bf16: Wt[p, t2, 32j+d] = W[j, i(km), d, k(km)]
    Wt = np.empty((128, 32, 1024), np.float32)
    for j in range(Nc):
        Wt[:, :, 32 * j:32 * j + 32] = Wc[j][ikm, :, kkm]
    # Wd [128, 8, 4096] bf16: Wd[32*(j%4)+d, j//4, ik] = W[j, i_ik, d, k_ik]
    Wd = np.empty((128, 8, 4096), np.float32)
    for j in range(Nc):
        Wd[32 * (j % 4):32 * (j % 4) + 32, j // 4, :] = Wc[j][_IK_I, :, _IK_K].T
    # xT [128, 32, 64] bf16 (pre-scaled 1/Nc): xT[p, t2, b] = x[b, i(km), k(km)]/Nc
    xT = (xc[:, ikm, kkm] / Nc).transpose(1, 2, 0)
    # x2a [(c2,b), ik] bf16 (same data both halves)
    xa = xc[:, _IK_I, _IK_K]                           # [B, 4096]
    x2a = np.concatenate([xa, xa], axis=0)             # [128, 4096]
    # xkT [km%128, t2, (c2,b)] bf16: host-transposed km-major x for (B)
    xk = xc[:, _KM_I, _KM_K]                           # [B, 4096]
    x2k = np.concatenate([xk, xk], axis=0)             # [128(c2,b), 4096]
    xkT = x2k.T.reshape(32, 128, 128).transpose(1, 0, 2)  # [128, 32, 128]

    # Wt0: m-major permutation of Wt (SBUF-resident on device; contiguous
    # 8KB-per-partition m-slices for the load)
    Wt0 = Wt.reshape(128, 32, 8, 128).transpose(0, 2, 1, 3)

    bf = ml_dtypes.bfloat16
    return {
        "Wt0": np.ascontiguousarray(Wt0.astype(bf)),
        "Wd": np.ascontiguousarray(Wd.astype(bf)),
        "xT": np.ascontiguousarray(xT.astype(bf)),
        "x2a": np.ascontiguousarray(x2a.astype(bf)),
        "xkT": np.ascontiguousarray(xkT.astype(bf)),
        "consts": _host_consts(),
    }


def _host_consts():
    """[128, 292] fp32 const block:
      cols [0:64]    hb: hb[p, b] = (p%64 == b) -- c2-half fold stationary
      cols [128:132] ones4 blockdiag: ones4[p, q] = (p//32 == q)
                     (partition-sum over d within a j-strip)
      cols [132:260] repM: repM[q, p] = (p//32 == q), used as [4, 128]
                     stationary to replicate a per-strip scalar over d
    """
    out = np.zeros((128, 292), np.float32)
    out[:, 290] = EPS                    # squash sqrt bias column
    for p in range(128):
        out[p, p % 64] = 1.0
    for q in range(4):
        out[32 * q:32 * q + 32, 128 + q] = 1.0
    for p in range(128):
        out[p // 32, 132 + p] = 1.0
    return np.ascontiguousarray(out)


def build_program(repeat=1, sim1=False):
    """Build the SPMD Bass/Tile program. repeat>1 duplicates the whole
    computation (for differential wall-clock timing). sim1=True builds a
    single-core variant (collective -> local DMA) for TimelineSim."""
    nc = bacc.Bacc("TRN2", target_bir_lowering=False, debug=False,
                   num_devices=(1 if sim1 else NCORES))

    d_Wt0 = nc.dram_tensor("Wt0", [128, 8, 32, 128], BF16,
                           kind="ExternalInput").ap()
    d_Wd = nc.dram_tensor("Wd", [128, 8, 4096], BF16, kind="ExternalInput").ap()
    d_xT = nc.dram_tensor("xT", [128, 32, 64], BF16, kind="ExternalInput").ap()
    d_x2a = nc.dram_tensor("x2a", [128, 4096], BF16, kind="ExternalInput").ap()
    d_xkT = nc.dram_tensor("xkT", [128, 32, 128], BF16,
                           kind="ExternalInput").ap()
    d_cst = nc.dram_tensor("consts", [128, 292], FP32, kind="ExternalInput").ap()
    d_out = nc.dram_tensor("out", [128, 8, 64], FP32, kind="ExternalOutput").ap()

    cc_in = [nc.dram_tensor(f"cc_in{h}", [128, 4, 64], BF16).ap()
             for h in range(2)]
    cc_out = [nc.dram_tensor(f"cc_out{h}", [128, 4, 64], BF16,
                             addr_space="Shared").ap() for h in range(2)]
    core_ids = list(range(NCORES))

    with tile.TileContext(nc) as tc, ExitStack() as ctx:
        ep = ctx.enter_context
        # ------------------------------------------------ pools
        p_const = ep(tc.tile_pool(name="const", bufs=1))
        p_wstream = ep(tc.tile_pool(name="wstream", bufs=6))
        p_small = ep(tc.tile_pool(name="small", bufs=1))
        p_gevac = ep(tc.tile_pool(name="gevac", bufs=3))
        p_prod = ep(tc.tile_pool(name="prod", bufs=2))
        p_red = ep(tc.tile_pool(name="red", bufs=1))
        p_eT = ep(tc.tile_pool(name="eT", bufs=4))
        p_xcT = ep(tc.tile_pool(name="xcT", bufs=4))
        # Single PSUM pool, one shared tag: slot = 4 banks, 2 slots = all 8.
        p_ps_g = ep(tc.tile_pool(name="ps_g", bufs=2, space="PSUM"))

        # ------------------------------------------------ resident tiles
        cst = p_const.tile([128, 292], FP32, tag="cst")
        nc.sync.dma_start(cst[:], d_cst)
        ones4 = cst[:, 128:132]          # [128, 4]: blockdiag over d-strips
        repM = cst[0:4, 132:260]         # [4, 128]: scale replicate stationary
        # bf16 copy of hb so it can be PE-stationary against bf16 moving data
        hb_bf = p_const.tile([128, 64], BF16, tag="hb_bf")
        nc.scalar.copy(hb_bf[:], cst[:, 0:64])

        xT = p_const.tile([128, 32, 64], BF16, tag="xT")
        x2a = p_const.tile([128, 4096], BF16, tag="x2a")
        xkT = p_const.tile([128, 32, 128], BF16, tag="xkT")
        # Startup: Wt_r owns the whole DMA pipe (pass0 is gated on its last
        # chunk). x2a/xkT are issued later from the DVE queue, behind the
        # pass0-h1 PSUM copy, so their transfers land after Wt completes --
        # they aren't consumed until (A)/(B) anyway.
        nc.sync.dma_start(xT[:], d_xT)
        Wt_r = p_const.tile([128, 8, 32, 128], BF16, tag="Wt_r")
        for m in range(8):
            # ACT queue (hardware DGE): keeps the 8MB load off the SP queue
            # so pass0's AllReduce copies aren't stuck behind it.
            nc.scalar.dma_start(Wt_r[:, m], d_Wt0[:, m])

        # bf16 logits: DVE reduce/add internal accum is fp32; one rounding per
        # pass. Keeps every (A) DVE op in the 2x perf mode and saves 8KB.
        logits = p_const.tile([128, 4, 4, 256], BF16, tag="logits")
        vT = p_const.tile([128, 8, 64], BF16, tag="vT")      # squash output
        # bf16 partial-sum exchange: the AllReduce payload halves and every
        # squash input read stays in the DVE 2x class. Costs one extra
        # rounding of the per-core partials (~4e-3 on s, well inside budget).
        sT_sb = p_const.tile([128, 8, 64], BF16, tag="sT_sb")
        e_t = p_const.tile([128, 16, 256], BF16, tag="e_t")  # exp(logits)

        for _rep in range(repeat):

            def allreduce_start(h):
                """Kick off AllReduce of slots [4h, 4h+4) of sT_sb: SBUF ->
                cc_in DMA + the collective. The return hop is emitted
                separately (allreduce_finish) so ready work -- next half's
                compute, hoisted (A) wd prefetches -- can be queued between
                the two without sitting behind the head-blocking return."""
                sl = np.s_[:, 4 * h:4 * h + 4, :]
                nc.sync.dma_start(cc_in[h], sT_sb[sl])
                if sim1:
                    nc.sync.dma_start(cc_out[h], cc_in[h])
                else:
                    nc.gpsimd.collective_compute(
                        "AllReduce", mybir.AluOpType.add,
                        replica_groups=[core_ids],
                        ins=[cc_in[h]], outs=[cc_out[h]],
                    )

            def allreduce_finish(h):
                sl = np.s_[:, 4 * h:4 * h + 4, :]
                nc.sync.dma_start(sT_sb[sl], cc_out[h])

            def squash(h, out_bf16, out_fp32=None):
                """Slots [4h, 4h+4) of sT_sb -> out_bf16 (+ fp32 copy).

                scale = s2/(1+s2) / sqrt(s2+eps); sqrt via ACT + one Newton
                step, divides via DVE bit-exact reciprocal.
                """
                sl = np.s_[:, 4 * h:4 * h + 4, :]
                sthl = sT_sb[sl]
                sq = p_small.tile([128, 4, 64], FP32, tag="sq")
                nc.vector.tensor_tensor(sq[:], sthl, sthl,
                                        op=mybir.AluOpType.mult)
                # one combined PSUM tile (1 bank) for both squash matmul
                # outputs: s2 block-sums in cols 64:128 (partitions 0:4),
                # the d-replicated scale in cols 0:64.
                ps_b = p_ps_g.tile([128, 4, 128], FP32, tag="ps")
                for slot in range(4):
                    nc.tensor.matmul(ps_b[0:4, slot, 64:128], ones4,
                                     sq[:, slot, :],
                                     start=True, stop=True,
                                     skip_group_check=True)
                # Short dependency chain (this sits on the AR critical path
                # 3x per kernel): y = sqrt(s2+eps) with eps folded into the
                # ACT bias; s2 read straight from PSUM everywhere; den
                # computed in parallel with y.
                s2p = ps_b[0:4, :, 64:128]
                y = p_small.tile([4, 4, 64], FP32, tag="y")
                nc.scalar.activation(y[:], s2p,
                                     mybir.ActivationFunctionType.Sqrt,
                                     bias=cst[0:4, 290:291])
                den = p_small.tile([4, 4, 64], FP32, tag="den")
                nc.vector.tensor_scalar(den[:], s2p, 1.0, None,
                                        op0=mybir.AluOpType.add)
                nc.vector.tensor_tensor(den[:], den[:], y[:], op=mybir.AluOpType.mult)
                nc.vector.reciprocal(den[:], den[:])
                scl = p_small.tile([4, 4, 64], FP32, tag="scl")
                nc.vector.tensor_tensor(scl[:], den[:], s2p, op=mybir.AluOpType.mult)
                # replicate over d: ps_b cols 0:64 = repM^T . scl
                for slot in range(4):
                    nc.tensor.matmul(ps_b[:, slot, 0:64], repM, scl[:, slot, :],
                                     start=True, stop=True,
                                     skip_group_check=True)
                if out_bf16 is not None:
                    nc.vector.tensor_tensor(out_bf16[sl], sthl,
                                            ps_b[:, :, 0:64],
                                            op=mybir.AluOpType.mult)
                if out_fp32 is not None:
                    nc.vector.tensor_tensor(out_fp32[sl], sthl,
                                            ps_b[:, :, 0:64],
                                            op=mybir.AluOpType.mult)

            def emit_A_pe(ga, cha, dma_eng=None, gate=None, direct=False):
                """(A) unit PE+ACT half: Wd stream, G matmuls, evacuation.
                Returns the gev tile for the DVE half. dma_eng picks the wd
                DMA queue; `gate` (an AP) adds a 1-elem marker copy into the
                wd tile first, forcing the DMA to request the pipe only
                after the gate's producer -- the scheduler can't hoist it."""
                wd_t = p_wstream.tile([128, 2, 1024], BF16, tag="wd_s")
                if gate is not None:
                    nc.gpsimd.tensor_copy(wd_t[0:1, 0, 0:1], gate)
                (dma_eng or nc.sync).dma_start(
                    wd_t[:], d_Wd[:, 2 * ga:2 * ga + 2,
                                  1024 * cha:1024 * cha + 1024])
                gev = None
                out_ps = []
                for chl in range(2):
                    ps_G = p_ps_g.tile([128, 4, 512], FP32, tag="ps")
                    for r in range(4):
                        for c2 in range(2):
                            nc.tensor.matmul(
                                ps_G[64 * c2:64 * c2 + 64, r, :],
                                vT[32 * r:32 * r + 32, 2 * ga + c2, :],
                                wd_t[32 * r:32 * r + 32, c2,
                                     512 * chl:512 * chl + 512],
                                start=True, stop=True,
                                tile_position=(32 * r, 64 * c2),
                            )
                    if direct:
                        # PSUM-direct unit: DVE reads G straight from PSUM
                        # (1x mode) -- trades DVE cycles for skipping the
                        # ACT evacuation, the (A) phase bottleneck.
                        out_ps.append(ps_G)
                        continue
                    # evac both chunk-halves into one double-width
                    # buffer; DVE then runs one wide unit per cha.
                    if chl == 0:
                        gev = p_gevac.tile([128, 4, 2, 512], BF16,
                                           tag="gev")
                    if pool_evac:
                        nc.gpsimd.tensor_copy(gev[:, :, chl, :], ps_G[:])
                    else:
                        nc.scalar.copy(gev[:, :, chl, :], ps_G[:])
                return out_ps if direct else gev

            hoisted = {}

            # ================================================ pass 0
            wt_gate = Wt_r[0:1, 7, 31, 127:128]   # last Wt chunk marker
            # sT0[jd, m, b] accumulated over 32 km-tiles directly in the
            # transposed layout: stationary = streamed Wt chunk [128, 128],
            # moving = xT (pre-scaled 1/Nc on host). No transposes needed,
            # and each 4-slot half goes to AllReduce while the other half
            # is still accumulating.
            for h in range(2):
                ps_sT0 = p_ps_g.tile([128, 4, 64], FP32, tag="ps")
                for ml in range(4):
                    m = 4 * h + ml
                    for t2 in range(32):
                        nc.tensor.matmul(
                            ps_sT0[:, ml, :], Wt_r[:, m, t2, :], xT[:, t2, :],
                            start=(t2 == 0), stop=(t2 == 31),
                            skip_group_check=True,
                        )
                sl = np.s_[:, 4 * h:4 * h + 4, :]
                nc.vector.tensor_copy(sT_sb[sl], ps_sT0[:])
                allreduce_start(h)
                if h == 1:
                    # (A)-pass-1 head start in this AllReduce's shadow
                    # (needs only vT-h0, squashed in the h=0 iteration).
                    # wd rides SP, each gated on the last Wt chunk so the
                    # stream starts only after Wt lands; xkT (needed only
                    # at (B)) comes last.
                    for u_h in range(6):
                        hoisted[(0, u_h // 4, u_h % 4)] = \
                            emit_A_pe(u_h // 4, u_h % 4)
                    nc.sync.dma_start(xkT[:], d_xkT)
                allreduce_finish(h)
                if h == 0:
                    # x2a load on SP after the AR-h0 hops: a 1-elem Pool
                    # gate (RAW on Wt-m7, WAW with the load) keeps its
                    # transfer from stealing DMA bandwidth from the
                    # critical Wt stream; emission after the return DMA
                    # keeps its issue-wait off the AR-h0 hop path.
                    nc.sync.dma_start(x2a[:], d_x2a)
                # fold of 1/Nc uniform-c scale: xT pre-scaled on host.
                squash(h, vT)

            # ================================================ passes 1, 2
            for pas in range(2):
                zp = p_small.tile([128, 4, 256], BF16, tag="zp")
                # ---------------- (A): G = Wd . vT ; logits += sum_k x2a * G
                for ga in range(4):
                    for cha in range(4):
                        res = hoisted.pop((pas, ga, cha), None)
                        # PSUM-direct units: ACT is the (A)-phase
                        # bottleneck (evac), so a fraction of units skip
                        # evacuation and multiply from PSUM on DVE at 1x.
                        direct = res is None and (ga, cha) in DIRECT_UNITS
                        if res is None:
                            res = emit_A_pe(ga, cha, direct=direct)
                        prod = p_prod.tile([128, 4, 1024], BF16, tag="prod")
                        x2sl = x2a[:, 1024 * cha:1024 * cha + 1024]
                        if direct:
                            for chl, psg in enumerate(res):
                                x2h = x2sl[:, 512 * chl:512 * chl + 512]
                                nc.vector.tensor_tensor(
                                    prod[:, :, 512 * chl:512 * chl + 512],
                                    psg[:],
                                    x2h.unsqueeze(1).broadcast_to(
                                        (128, 4, 512)),
                                    op=mybir.AluOpType.mult)
                        else:
                            gev = res
                            # DVE is the busiest engine; Pool (gpsimd)
                            # idles. Pool runs ~3.8x slower per row, so it
                            # gets the r=3 quarter of every unit op — both
                            # engines then finish their share of each unit
                            # at similar times.
                            gv = gev[:].rearrange("p r c f -> p r (c f)")
                            x2b = x2sl.unsqueeze(1).broadcast_to(
                                (128, 4, 1024))
                            for eng, rs in ((nc.vector, slice(0, 3)),
                                            (nc.gpsimd, slice(3, 4))):
                                eng.tensor_tensor(
                                    prod[:, rs], gv[:, rs], x2b[:, rs],
                                    op=mybir.AluOpType.mult)
                        # TensorReduce has no 2x uop (1x only): sum k=16 as
                        # a log-tree of in-place TT adds, all 2x-mode.
                        pv = prod[:].rearrange("p r (i k) -> p r i k", k=16)
                        lsl = logits[:, ga, :, 64 * cha:64 * cha + 64]
                        for eng, rs, ws in ((nc.vector, slice(0, 3), (8, 4, 2)),
                                            (nc.gpsimd, slice(3, 4), (8, 4))):
                            for w in ws:
                                eng.tensor_tensor(
                                    pv[:, rs, :, 0:w], pv[:, rs, :, 0:w],
                                    pv[:, rs, :, w:2 * w],
                                    op=mybir.AluOpType.add)
                        # r=3 w2 level back on DVE (Pool runs ~3.8x slower)
                        nc.vector.tensor_tensor(
                            pv[:, 3:4, :, 0:2], pv[:, 3:4, :, 0:2],
                            pv[:, 3:4, :, 2:4], op=mybir.AluOpType.add)
                        # last tree level fused with the logits update
                        if pas == 0:
                            nc.vector.tensor_tensor(
                                lsl, pv[:, :, :, 0], pv[:, :, :, 1],
                                op=mybir.AluOpType.add)
                        else:
                            red = p_red.tile([128, 4, 64], BF16, tag="red")
                            nc.vector.tensor_tensor(
                                red[:], pv[:, :, :, 0], pv[:, :, :, 1],
                                op=mybir.AluOpType.add)
                            nc.vector.tensor_tensor(lsl, lsl, red[:],
                                                    op=mybir.AluOpType.add)
                    # per-ga exp + Z partial: hidden under the next ga's
                    # (A) work; only ga=3's tail sits on the critical path.
                    nc.scalar.activation(
                        e_t[:, 4 * ga:4 * ga + 4, :].rearrange(
                            "p a b -> p (a b)"),
                        logits[:, ga].rearrange("p r i -> p (r i)"),
                        mybir.ActivationFunctionType.Exp)
                    zl1 = p_red.tile([128, 2, 256], BF16, tag="zl1")
                    nc.vector.tensor_tensor(
                        zl1[:], e_t[:, 4 * ga:4 * ga + 2, :],
                        e_t[:, 4 * ga + 2:4 * ga + 4, :],
                        op=mybir.AluOpType.add)
                    nc.vector.tensor_tensor(zp[:, ga, :], zl1[:, 0, :],
                                            zl1[:, 1, :],
                                            op=mybir.AluOpType.add)
                    if ga == 0:
                        # prefetch (B)-m0's eT transposes: their DMA latency
                        # hides under the remaining (A) work.
                        eT_pre = []
                        for jj in range(4):
                            eTp = p_eT.tile([128, 2, 128], BF16, tag="eT")
                            nc.sync.dma_start(eTp[:], e_t[:, jj, :],
                                              transpose=True)
                            eT_pre.append(eTp)
                # ---------------- softmax tail: fold Z partials, 1/Z,
                # transpose into the (B) km-partition layout.
                zq2 = p_small.tile([128, 2, 256], BF16, tag="zq2")
                nc.vector.tensor_tensor(zq2[:], zp[:, 0:2, :], zp[:, 2:4, :],
                                        op=mybir.AluOpType.add)
                # c2-halves + plane fold on PE: two matmuls accumulate both
                # zq2 planes into one PSUM region -> Z[b, i]
                ps_Z = p_ps_g.tile([64, 256], FP32, tag="ps")
                for pl in range(2):
                    nc.tensor.matmul(ps_Z[:], hb_bf, zq2[:, pl, :],
                                     start=(pl == 0), stop=(pl == 1))
                rz64 = p_small.tile([64, 256], BF16, tag="rz64")
                with nc.allow_low_precision("bf16 softmax 1/Z"):
                    nc.vector.reciprocal(rz64[:], ps_Z[:])
                # Block-transpose: rzT[p, ib, b] = rz64[b, 128*ib+p] = 1/Z[b,i]
                rzT = p_small.tile([128, 2, 64], BF16, tag="rzT")
                nc.sync.dma_start(rzT[:], rz64[:], transpose=True)
                # xrT[p, (k,ib), (c2,b)] = xkT * rzT (bcast over k), one
                # 4D op per c2 half (ISA APs allow only 3 free dims)
                xrT = p_small.tile([128, 32, 128], BF16, tag="xrT")
                xrv = xrT[:].rearrange("p (k ib) n -> p k ib n", ib=2)
                xkv = xkT[:].rearrange("p (k ib) n -> p k ib n", ib=2)
                rzv = rzT[:].unsqueeze(1).broadcast_to((128, 16, 2, 64))
                for c2h in range(2):
                    nsl = np.s_[:, :, :, 64 * c2h:64 * c2h + 64]
                    for eng, ks in ((nc.vector, slice(0, 14)),
                                    (nc.gpsimd, slice(14, 16))):
                        eng.tensor_tensor(xrv[nsl][:, ks], xkv[nsl][:, ks],
                                          rzv[:, ks],
                                          op=mybir.AluOpType.mult)
                # ---------------- (B): xcT = xrT * e_jj^T -> PE contraction
                last = (pas == 1)
                if last:
                    vfin = p_small.tile([128, 8, 64], FP32, tag="vfin")
                ps_sT = None
                for m in range(4):
                    if m % 2 == 0:
                        ps_sT = p_ps_g.tile([128, 4, 64], FP32, tag="ps")
                    xcT_bufs = []
                    for jq in range(4):
                        jj = 4 * m + jq
                        if m == 0:
                            eT = eT_pre[jq]
                        else:
                            eT = p_eT.tile([128, 2, 128], BF16, tag="eT")
                            # ACT queue: idle during (B); keeps SP free for
                            # the AllReduce hops.
                            nc.scalar.dma_start(eT[:], e_t[:, jj, :],
                                                transpose=True)
                        xcT = p_xcT.tile([128, 32, 128], BF16, tag="xcT")
                        # k-split: DVE takes 13/16 of rows, Pool 3/16
                        xv = xcT[:].rearrange("p (k ib) n -> p k ib n", ib=2)
                        rv = xrT[:].rearrange("p (k ib) n -> p k ib n", ib=2)
                        ev = eT[:].unsqueeze(1).broadcast_to((128, 16, 2, 128))
                        for eng, ks in ((nc.vector, slice(0, 13)),
                                        (nc.gpsimd, slice(13, 16))):
                            eng.tensor_tensor(
                                xv[:, ks], rv[:, ks], ev[:, ks],
                                op=mybir.AluOpType.mult)
                        xcT_bufs.append(xcT)
                    # t2 INNERMOST, q cycling fastest across groups: each
                    # accumulation group runs start->stop contiguously, and
                    # consecutive groups sit on different col-tiles so the
                    # array overlaps them 4-wide on HW.
                    for gq in (2 * m, 2 * m + 1):
                        c2 = gq % 2
                        for q in range(4):
                            j = 4 * gq + q           # j%4 == q, jj = 4*m + q
                            jl = j - 8 * m
                            for t2 in range(32):
                                nc.tensor.matmul(
                                    ps_sT[32 * q:32 * q + 32, gq % 4, :],
                                    Wt_r[:, 2 * m + jl // 4, t2,
                                         32 * (jl % 4):32 * (jl % 4) + 32],
                                    xcT_bufs[q][:, t2, 64 * c2:64 * c2 + 64],
                                    start=(t2 == 0), stop=(t2 == 31),
                                    tile_position=(0, 32 * q),
                                    skip_group_check=True,
                                )
                    # Kick each half's AllReduce as soon as its two m-blocks
                    # land; returns + squash are deferred below so no DVE/SP
                    # wait sits ahead of still-ready (B) work.
                    if m % 2 == 1:
                        h = m // 2
                        sl = np.s_[:, 4 * h:4 * h + 4, :]
                        nc.vector.tensor_copy(sT_sb[sl], ps_sT[:])
                        allreduce_start(h)
                # Tail: AR returns + squash. All xcT/PE work is already
                # queued, so the DVE queue blocking on the h0 return sem
                # no longer head-blocks m2/m3 work (16us stall in the
                # baseline trace).
                for h in range(2):
                    sl = np.s_[:, 4 * h:4 * h + 4, :]
                    allreduce_finish(h)
                    if not last:
                        squash(h, vT)
                        if h == 0:
                            # (A)-pass+1 head start in AR-h1's shadow (needs
                            # only vT-h0 = slots 0-3, covering ga=0 and 1).
                            # ga=1's direct-type units stay unhoisted.
                            for hga, hcha in ((0, 0), (0, 1), (0, 2), (0, 3),
                                              (1, 0), (1, 1), (1, 3)):
                                hoisted[(pas + 1, hga, hcha)] = \
                                    emit_A_pe(hga, hcha)
                    else:
                        # vT is dead after the final pass: only emit the
                        # fp32 output product. Out-DMA rides ACT (idle now).
                        squash(h, None, out_fp32=vfin)
                        nc.scalar.dma_start(d_out[sl], vfin[sl])

    nc.compile()
    return nc


def kernel(x, W):
    x = np.asarray(x, dtype=np.float32)
    W = np.asarray(W, dtype=np.float32)
    in_maps = [_host_prep_core(x, W, c) for c in range(NCORES)]

    nc = build_program()
    res = run_bass_kernel_spmd(nc, in_maps, list(range(NCORES)))
    vT = res.results[0]["out"]  # [128, 8, 64]

    v = np.empty((B, Nc, Dc), np.float32)
    for j in range(Nc):
        v[:, j, :] = vT[32 * (j % 4):32 * (j % 4) + 32, j // 4, :].T
    return v


if __name__ == "__main__":
    rng = np.random.default_rng(0)
    x = rng.standard_normal((B, In, Din), dtype=np.float32)
    W = (rng.standard_normal((Nc, In, Dc, Din), dtype=np.float32) * 0.05)
    out = kernel(x, W)
    print("kernel ran; out shape", out.shape, "mean", float(np.abs(out).mean()))

